# revision 13
# baseline (speedup 1.0000x reference)
"""Trainium2 Bass kernel for nn_CausalLinearSelfAttention_30013231464545.

Math note: the reference cumsums the [B,T,H,D,M] kv tensor over axis=-2,
which is the *D* axis (faithful to the original torch code), so
  kv_sum[b,t,h,d,m] = csD(kf)[b,t,h,d] * v[b,t,h,m]
and the whole module collapses to
  out[b,t,h,m] = (s / denom) * v[b,t,h,m]
with
  denom[b,t,h] = sum_d qf * cumsum_T(kf)      (true causal running key sum)
  s[b,t,h]     = sum_d qf * cumsum_D(kf)      (per-timestep D-prefix sum)
  qf = elu(q)+1 = min(exp(q), 1) + relu(q),  kf likewise.

Sharding: B*H = 16 (b,h) slices; each core takes one (b, head-pair) slice
[T=2048, 2*64] so DMA rows are 512B contiguous. No cross-core comm.

Per-core dataflow, v2 (T on partitions, (j,h,d) on free; fp16 on-chip):
  - 2 half-tensor chunks pipelined against the input DMA stream
  - feature maps: ACT exp (scalar) + fused (min 1) add relu via one
    scalar_tensor_tensor (DVE)
  - cumsum over T: per-tile triangular matmuls; the inter-tile carry runs
    entirely on the tensor engine (colsum matmuls onto partitions,
    one exclusive-tri16 matmul, rank-1 base adds) - no serial DVE chain
  - scalar engine copies PSUM ks -> SBUF f16 so the dot mults stay in
    DVE 2x fast mode
  - cumsum over D: one segmented scan per chunk (GpSimd, off the DVE)
  - dots over D: two f16 TT mults + one multi-axis tensor_reduce
  - out = v * (s/denom) broadcast multiply, DMA'd out per chunk
"""

import numpy as np
import sys

sys.path.insert(0, "/opt/trn_rl_repo")

B, T, H, D = 2, 2048, 8, 64
P = 128          # partitions (t per tile)
HPC = 2          # heads per core
C = HPC * D      # per-core free width = 128
NT = T // P      # 16 t-tiles per core
NCH = 2          # pipeline chunks
JT = NT // NCH   # 8 tiles per chunk
FD = JT * C      # 1024 free elements per chunk
NSEG = JT * HPC  # 16 (tile, head) segments per chunk

USE_GPSIMD_SCAN = False  # compiler rejects TensorTensorScan on the Pool engine

_CACHE = {}


def _build_nc():
    import concourse.bass as bass
    import concourse.bacc as bacc
    import concourse.mybir as mybir
    from concourse import tile

    dt = mybir.dt
    f32 = dt.float32
    f16 = dt.float16
    Alu = mybir.AluOpType
    Act = mybir.ActivationFunctionType

    nc = bacc.Bacc(None)

    q_d = nc.declare_dram_parameter("q", [T, C], f32, isOutput=False)
    k_d = nc.declare_dram_parameter("k", [T, C], f32, isOutput=False)
    v_d = nc.declare_dram_parameter("v", [T, C], f32, isOutput=False)
    o_d = nc.declare_dram_parameter("o", [T, C], f32, isOutput=True)

    # tri[t', t] = 1 if t' <= t  (lhsT for in-tile cumsum over partitions)
    tri_d = nc.inline_tensor(
        np.triu(np.ones((P, P), dtype=np.float16)), name="tri_const"
    )
    # gate[p, (j, c)] = 1 if p < j: masks tile-colsums for the base matmul
    gate_np = np.zeros((NT, NT, C), dtype=np.float16)
    for j in range(NT):
        gate_np[:j, j, :] = 1.0
    gate_d = nc.inline_tensor(gate_np.reshape(NT, NT * C), name="gate_const")
    # colsel[:, j, m] = (m == j): drops tile j's colsum onto psum row j
    colsel_np = np.zeros((P, JT, JT), dtype=np.float16)
    for j in range(JT):
        colsel_np[:, j, j] = 1.0
    colsel_d = nc.inline_tensor(colsel_np.reshape(P, JT * JT), name="colsel_const")

    with tile.TileContext(nc) as tc:
        with (
            tc.tile_pool(name="const", bufs=1) as cpool,
            tc.tile_pool(name="io", bufs=2) as io,
            tc.tile_pool(name="wk", bufs=2) as wk,
            tc.tile_pool(name="ps", bufs=2, space="PSUM") as pp,
            tc.tile_pool(name="pcs", bufs=1, space="PSUM") as pcs,
        ):
            # ---- input DMAs first: chunked, in consumption order ----
            qtw = io.tile([P, T], f32, tag="q")
            ktw = io.tile([P, T], f32, tag="k")
            vtw = io.tile([P, T], f32, tag="v")

            def load_chunk(tw, d_, ch):
                rows = slice(ch * JT * P, (ch + 1) * JT * P)
                nc.sync.dma_start(
                    tw[:, ch * FD : (ch + 1) * FD].rearrange(
                        "p (j c) -> p j c", c=C
                    ),
                    d_[rows, :].rearrange("(j p) c -> p j c", p=P),
                )

            for ch in range(NCH):
                load_chunk(ktw, k_d, ch)
                load_chunk(qtw, q_d, ch)
            for ch in range(NCH):
                load_chunk(vtw, v_d, ch)

            # ---- constants ----
            tri_t = cpool.tile([P, P], f16, tag="tri")
            nc.sync.dma_start(tri_t[:], tri_d[:])
            gate_t = cpool.tile([NT, NT * C], f16, tag="gate")
            nc.sync.dma_start(gate_t[:], gate_d[:])
            colsel_t = cpool.tile([P, JT * JT], f16, tag="colsel")
            nc.sync.dma_start(colsel_t[:], colsel_d[:])
            onesK = cpool.tile([NT, P], f16, tag="onesK")
            nc.vector.memset(onesK[:], 1.0)
            # segmented-scan reset mask: 0 at the first column of each 64-seg
            mask_t = cpool.tile([P, FD], f16, tag="mask")
            nc.vector.memset(mask_t[:], 1.0)
            nc.vector.memset(
                mask_t[:].rearrange("p (s d) -> p s d", d=D)[:, :, 0:1], 0.0
            )
            # per-tile colsum rows for chunk>0, spread over partitions
            csm = cpool.tile([NT, C], f16, tag="csm")
            cst = [None] * NCH

            scan_eng = nc.gpsimd if USE_GPSIMD_SCAN else nc.vector

            for ch in range(NCH):
                gsl = slice(ch * FD, (ch + 1) * FD)
                kt = ktw[:, gsl]
                qt = qtw[:, gsl]
                vt = vtw[:, gsl]

                # ---- feature maps: f = min(exp(x),1) + relu(x) ----
                ek = wk.tile([P, FD], f16, tag="ek")
                nc.scalar.activation(ek[:], kt, Act.Exp)
                rk = wk.tile([P, FD], f16, tag="rk")
                nc.gpsimd.tensor_scalar_max(rk[:], kt, 0.0)
                kf = wk.tile([P, FD], f16, tag="kf")
                nc.vector.scalar_tensor_tensor(
                    kf[:], ek[:], 1.0, rk[:], op0=Alu.min, op1=Alu.add
                )

                eq = wk.tile([P, FD], f16, tag="eq")
                nc.scalar.activation(eq[:], qt, Act.Exp)
                rq = wk.tile([P, FD], f16, tag="rq")
                nc.gpsimd.tensor_scalar_max(rq[:], qt, 0.0)
                qf = wk.tile([P, FD], f16, tag="qf")
                nc.vector.scalar_tensor_tensor(
                    qf[:], eq[:], 1.0, rq[:], op0=Alu.min, op1=Alu.add
                )

                # ---- per-tile colsums onto psum rows 0..JT-1 ----
                cs_ps = pcs.tile([JT, C], f32, tag="cs")
                for j in range(JT):
                    nc.tensor.matmul(
                        cs_ps[:],
                        colsel_t[:, j * JT : (j + 1) * JT],
                        kf[:, j * C : (j + 1) * C],
                        start=(j == 0), stop=(j == JT - 1),
                    )
                cst_ch = wk.tile([JT, C], f16, tag=f"cst{ch}")
                cst[ch] = cst_ch
                nc.scalar.copy(cst_ch[:], cs_ps[:])
                # stage rows into csm (only chunk>0 reads it)
                nc.sync.dma_start(csm[ch * JT : (ch + 1) * JT, :], cst[ch][:])

                # gated colsums: rhs[p, (j, c)] = csm[p, c] * [p < jg]
                nk = (ch + 1) * JT
                gsl_j = slice(ch * JT * C, (ch + 1) * JT * C)
                rhsm = wk.tile([NT, JT * C], f16, tag="rhsm")
                cs_src = cst[0][:] if ch == 0 else csm[0:nk, :]
                nc.vector.tensor_tensor(
                    rhsm[0:nk, :].rearrange("p (j c) -> p j c", c=C),
                    cs_src.rearrange(
                        "p (one c) -> p one c", one=1
                    ).broadcast_to([nk, JT, C]),
                    gate_t[0:nk, gsl_j].rearrange("p (j c) -> p j c", c=C),
                    op=Alu.mult,
                )

                # ---- cumsum over T into PSUM: tri matmuls + one base mm ----
                ks = pp.tile([P, FD], f32, tag="ks")
                # start=True resets the whole PSUM bank, so only the first
                # write per 512-f32 bank may set it
                for j in range(JT):
                    sl = slice(j * C, (j + 1) * C)
                    nc.tensor.matmul(
                        ks[:, sl], tri_t[:], kf[:, sl],
                        start=(j % 4 == 0), stop=False,
                    )
                ksf = wk.tile([P, FD], f16, tag="ksf")
                for half in range(FD // 512):
                    hs = slice(half * 512, (half + 1) * 512)
                    nc.tensor.matmul(
                        ks[:, hs], onesK[0:nk, :], rhsm[0:nk, hs],
                        start=False, stop=True,
                    )
                    nc.scalar.copy(ksf[:, hs], ks[:, hs])

                # ---- cumsum over D: one segmented scan ----
                csDm = wk.tile([P, FD], f16, tag="csDm")
                scan_eng.tensor_tensor_scan(
                    csDm[:], mask_t[:], kf[:], 0.0, op0=Alu.mult, op1=Alu.add
                )

                # ---- dots over D (s-part first: ready before ksf) ----
                scr = wk.tile([P, 2 * FD], f16, tag="scr")
                dn = wk.tile([P, 2 * NSEG], f32, tag="dn")
                nc.vector.tensor_tensor(
                    scr[:, FD : 2 * FD], qf[:], csDm[:], op=Alu.mult
                )
                nc.vector.tensor_reduce(
                    dn[:, NSEG : 2 * NSEG],
                    scr[:, FD : 2 * FD].rearrange("p (s d) -> p s d", d=D),
                    axis=mybir.AxisListType.X,
                    op=Alu.add,
                )
                nc.vector.tensor_tensor(
                    scr[:, 0:FD], qf[:], ksf[:], op=Alu.mult
                )
                nc.vector.tensor_reduce(
                    dn[:, 0:NSEG],
                    scr[:, 0:FD].rearrange("p (s d) -> p s d", d=D),
                    axis=mybir.AxisListType.X,
                    op=Alu.add,
                )

                # scale = s / denom
                rec = wk.tile([P, NSEG], f32, tag="rec")
                nc.vector.reciprocal(rec[:], dn[:, 0:NSEG])
                sc = wk.tile([P, NSEG], f32, tag="sc")
                nc.vector.tensor_tensor(
                    sc[:], dn[:, NSEG : 2 * NSEG], rec[:], op=Alu.mult
                )

                # out = v * scale (broadcast over each 64-wide segment)
                ot = io.tile([P, FD], f32, tag="o")
                sc_b = sc[:].rearrange(
                    "p (s one) -> p s one", one=1
                ).broadcast_to([P, NSEG, D])
                nc.vector.tensor_tensor(
                    ot[:].rearrange("p (s d) -> p s d", d=D),
                    vt.rearrange("p (s d) -> p s d", d=D),
                    sc_b,
                    op=Alu.mult,
                )
                rows = slice(ch * JT * P, (ch + 1) * JT * P)
                ov = o_d[rows, :].rearrange("(j p) c -> p j c", p=P)
                nc.sync.dma_start(ov, ot[:].rearrange("p (j c) -> p j c", c=C))

    nc.compile()
    return nc


def get_nc():
    if "nc" not in _CACHE:
        _CACHE["nc"] = _build_nc()
    return _CACHE["nc"]


def shard_inputs(q, k, v):
    """core c -> (b = c//4, heads 2*(c%4), 2*(c%4)+1); returns list of in_maps."""
    maps = []
    for c in range(8):
        b, hp = divmod(c, 4)
        hs = slice(2 * hp, 2 * hp + 2)
        maps.append(
            {
                "q": np.ascontiguousarray(q[b, :, hs, :].reshape(T, C)),
                "k": np.ascontiguousarray(k[b, :, hs, :].reshape(T, C)),
                "v": np.ascontiguousarray(v[b, :, hs, :].reshape(T, C)),
            }
        )
    return maps


def gather_outputs(results):
    out = np.empty((B, T, H, D), dtype=np.float32)
    for c in range(8):
        b, hp = divmod(c, 4)
        out[b, :, 2 * hp : 2 * hp + 2, :] = results[c]["o"].reshape(T, HPC, D)
    return out


def kernel(q, k, v):
    from concourse.bass_utils import run_bass_kernel_spmd

    q = np.asarray(q, dtype=np.float32)
    k = np.asarray(k, dtype=np.float32)
    v = np.asarray(v, dtype=np.float32)
    nc = get_nc()
    maps = shard_inputs(q, k, v)
    res = run_bass_kernel_spmd(nc, maps, list(range(8)))
    return gather_outputs(res.results)


# revision 14
# speedup vs baseline: 2.2108x; 2.2108x over previous
"""Trainium2 Bass kernel for nn_CausalLinearSelfAttention_30013231464545.

Math note: the reference cumsums the [B,T,H,D,M] kv tensor over axis=-2,
which is the *D* axis (faithful to the original torch code), so
  kv_sum[b,t,h,d,m] = csD(kf)[b,t,h,d] * v[b,t,h,m]
and the whole module collapses to
  out[b,t,h,m] = (s / denom) * v[b,t,h,m]
with
  denom[b,t,h] = sum_d qf * cumsum_T(kf)      (true causal running key sum)
  s[b,t,h]     = sum_d qf * cumsum_D(kf)      (per-timestep D-prefix sum)
  qf = elu(q)+1 = min(exp(q), 1) + relu(q),  kf likewise.

Sharding: B*H = 16 (b,h) slices; each core takes one (b, head-pair) slice
[T=2048, 2*64] so DMA rows are 512B contiguous. No cross-core comm.

Per-core dataflow, v2 (T on partitions, (j,h,d) on free; fp16 on-chip):
  - 2 half-tensor chunks pipelined against the input DMA stream
  - feature maps: ACT exp (scalar) + fused (min 1) add relu via one
    scalar_tensor_tensor (DVE)
  - cumsum over T: per-tile triangular matmuls; the inter-tile carry runs
    entirely on the tensor engine (colsum matmuls onto partitions,
    one exclusive-tri16 matmul, rank-1 base adds) - no serial DVE chain
  - scalar engine copies PSUM ks -> SBUF f16 so the dot mults stay in
    DVE 2x fast mode
  - cumsum over D: one segmented scan per chunk (GpSimd, off the DVE)
  - dots over D: two f16 TT mults + one multi-axis tensor_reduce
  - out = v * (s/denom) broadcast multiply, DMA'd out per chunk
"""

import numpy as np
import sys

sys.path.insert(0, "/opt/trn_rl_repo")

B, T, H, D = 2, 2048, 8, 64
P = 128          # partitions (t per tile)
HPC = 2          # heads per core
C = HPC * D      # per-core free width = 128
NT = T // P      # 16 t-tiles per core
NCH = 2          # pipeline chunks
JT = NT // NCH   # 8 tiles per chunk
FD = JT * C      # 1024 free elements per chunk
NSEG = JT * HPC  # 16 (tile, head) segments per chunk

USE_GPSIMD_SCAN = False  # compiler rejects TensorTensorScan on the Pool engine

_CACHE = {}


def _build_nc():
    import concourse.bass as bass
    import concourse.bacc as bacc
    import concourse.mybir as mybir
    from concourse import tile

    dt = mybir.dt
    f32 = dt.float32
    f16 = dt.float16
    Alu = mybir.AluOpType
    Act = mybir.ActivationFunctionType

    nc = bacc.Bacc(None)

    q_d = nc.declare_dram_parameter("q", [T, C], f32, isOutput=False)
    k_d = nc.declare_dram_parameter("k", [T, C], f32, isOutput=False)
    v_d = nc.declare_dram_parameter("v", [T, C], f32, isOutput=False)
    o_d = nc.declare_dram_parameter("o", [T, C], f32, isOutput=True)

    # tri[t', t] = 1 if t' <= t  (lhsT for in-tile cumsum over partitions)
    tri_d = nc.inline_tensor(
        np.triu(np.ones((P, P), dtype=np.float16)), name="tri_const"
    )
    # gate[p, (j, c)] = 1 if p < j: masks tile-colsums for the base matmul
    gate_np = np.zeros((NT, NT, C), dtype=np.float16)
    for j in range(NT):
        gate_np[:j, j, :] = 1.0
    gate_d = nc.inline_tensor(gate_np.reshape(NT, NT * C), name="gate_const")
    # colsel[:, j, m] = (m == j): drops tile j's colsum onto psum row j
    colsel_np = np.zeros((P, JT, JT), dtype=np.float16)
    for j in range(JT):
        colsel_np[:, j, j] = 1.0
    colsel_d = nc.inline_tensor(colsel_np.reshape(P, JT * JT), name="colsel_const")

    with tile.TileContext(nc) as tc:
        with (
            tc.tile_pool(name="const", bufs=1) as cpool,
            tc.tile_pool(name="io", bufs=2) as io,
            tc.tile_pool(name="wk", bufs=2) as wk,
            tc.tile_pool(name="ps", bufs=2, space="PSUM") as pp,
            tc.tile_pool(name="pcs", bufs=1, space="PSUM") as pcs,
        ):
            # ---- input DMAs first: chunked, in consumption order ----
            qtw = io.tile([P, T], f32, tag="q")
            ktw = io.tile([P, T], f32, tag="k")
            vtw = io.tile([P, T], f32, tag="v")

            def load_chunk(tw, d_, ch):
                rows = slice(ch * JT * P, (ch + 1) * JT * P)
                nc.sync.dma_start(
                    tw[:, ch * FD : (ch + 1) * FD].rearrange(
                        "p (j c) -> p j c", c=C
                    ),
                    d_[rows, :].rearrange("(j p) c -> p j c", p=P),
                )

            for ch in range(NCH):
                load_chunk(ktw, k_d, ch)
                load_chunk(qtw, q_d, ch)
            for ch in range(NCH):
                load_chunk(vtw, v_d, ch)

            # ---- constants ----
            tri_t = cpool.tile([P, P], f16, tag="tri")
            nc.sync.dma_start(tri_t[:], tri_d[:])
            gate_t = cpool.tile([NT, NT * C], f16, tag="gate")
            nc.sync.dma_start(gate_t[:], gate_d[:])
            colsel_t = cpool.tile([P, JT * JT], f16, tag="colsel")
            nc.sync.dma_start(colsel_t[:], colsel_d[:])
            onesK = cpool.tile([NT, P], f16, tag="onesK")
            nc.vector.memset(onesK[:], 1.0)
            # segmented-scan reset mask: 0 at the first column of each 64-seg
            mask_t = cpool.tile([P, FD], f16, tag="mask")
            nc.vector.memset(mask_t[:], 1.0)
            nc.vector.memset(
                mask_t[:].rearrange("p (s d) -> p s d", d=D)[:, :, 0:1], 0.0
            )
            # per-tile colsum rows for chunk>0, spread over partitions
            csm = cpool.tile([NT, C], f16, tag="csm")
            cst = [None] * NCH

            scan_eng = nc.gpsimd if USE_GPSIMD_SCAN else nc.vector

            for ch in range(NCH):
                gsl = slice(ch * FD, (ch + 1) * FD)
                kt = ktw[:, gsl]
                qt = qtw[:, gsl]
                vt = vtw[:, gsl]

                # ---- feature maps: f = min(exp(x),1) + relu(x) ----
                ek = wk.tile([P, FD], f16, tag="ek")
                nc.scalar.activation(ek[:], kt, Act.Exp)
                rk = wk.tile([P, FD], f16, tag="rk")
                nc.vector.tensor_scalar_max(rk[:], kt, 0.0)
                kf = wk.tile([P, FD], f16, tag="kf")
                nc.vector.scalar_tensor_tensor(
                    kf[:], ek[:], 1.0, rk[:], op0=Alu.min, op1=Alu.add
                )

                eq = wk.tile([P, FD], f16, tag="eq")
                nc.scalar.activation(eq[:], qt, Act.Exp)
                rq = wk.tile([P, FD], f16, tag="rq")
                nc.vector.tensor_scalar_max(rq[:], qt, 0.0)
                qf = wk.tile([P, FD], f16, tag="qf")
                nc.vector.scalar_tensor_tensor(
                    qf[:], eq[:], 1.0, rq[:], op0=Alu.min, op1=Alu.add
                )

                # ---- per-tile colsums onto psum rows 0..JT-1 ----
                cs_ps = pcs.tile([JT, C], f32, tag="cs")
                for j in range(JT):
                    nc.tensor.matmul(
                        cs_ps[:],
                        colsel_t[:, j * JT : (j + 1) * JT],
                        kf[:, j * C : (j + 1) * C],
                        start=(j == 0), stop=(j == JT - 1),
                    )
                cst_ch = wk.tile([JT, C], f16, tag=f"cst{ch}")
                cst[ch] = cst_ch
                nc.scalar.copy(cst_ch[:], cs_ps[:])
                # stage rows into csm (only chunk>0 reads it)
                nc.sync.dma_start(csm[ch * JT : (ch + 1) * JT, :], cst[ch][:])

                # gated colsums: rhs[p, (j, c)] = csm[p, c] * [p < jg]
                nk = (ch + 1) * JT
                gsl_j = slice(ch * JT * C, (ch + 1) * JT * C)
                rhsm = wk.tile([NT, JT * C], f16, tag="rhsm")
                cs_src = cst[0][:] if ch == 0 else csm[0:nk, :]
                nc.vector.tensor_tensor(
                    rhsm[0:nk, :].rearrange("p (j c) -> p j c", c=C),
                    cs_src.rearrange(
                        "p (one c) -> p one c", one=1
                    ).broadcast_to([nk, JT, C]),
                    gate_t[0:nk, gsl_j].rearrange("p (j c) -> p j c", c=C),
                    op=Alu.mult,
                )

                # ---- cumsum over T into PSUM: tri matmuls + one base mm ----
                ks = pp.tile([P, FD], f32, tag="ks")
                # start=True resets the whole PSUM bank, so only the first
                # write per 512-f32 bank may set it
                for j in range(JT):
                    sl = slice(j * C, (j + 1) * C)
                    nc.tensor.matmul(
                        ks[:, sl], tri_t[:], kf[:, sl],
                        start=(j % 4 == 0), stop=False,
                    )
                ksf = wk.tile([P, FD], f16, tag="ksf")
                for half in range(FD // 512):
                    hs = slice(half * 512, (half + 1) * 512)
                    nc.tensor.matmul(
                        ks[:, hs], onesK[0:nk, :], rhsm[0:nk, hs],
                        start=False, stop=True,
                    )
                    nc.scalar.copy(ksf[:, hs], ks[:, hs])

                # ---- cumsum over D: one segmented scan ----
                csDm = wk.tile([P, FD], f16, tag="csDm")
                scan_eng.tensor_tensor_scan(
                    csDm[:], mask_t[:], kf[:], 0.0, op0=Alu.mult, op1=Alu.add
                )

                # ---- dots over D (s-part first: ready before ksf) ----
                scr = wk.tile([P, 2 * FD], f16, tag="scr")
                dn = wk.tile([P, 2 * NSEG], f32, tag="dn")
                nc.vector.tensor_tensor(
                    scr[:, FD : 2 * FD], qf[:], csDm[:], op=Alu.mult
                )
                nc.vector.tensor_reduce(
                    dn[:, NSEG : 2 * NSEG],
                    scr[:, FD : 2 * FD].rearrange("p (s d) -> p s d", d=D),
                    axis=mybir.AxisListType.X,
                    op=Alu.add,
                )
                nc.vector.tensor_tensor(
                    scr[:, 0:FD], qf[:], ksf[:], op=Alu.mult
                )
                nc.vector.tensor_reduce(
                    dn[:, 0:NSEG],
                    scr[:, 0:FD].rearrange("p (s d) -> p s d", d=D),
                    axis=mybir.AxisListType.X,
                    op=Alu.add,
                )

                # scale = s / denom
                rec = wk.tile([P, NSEG], f32, tag="rec")
                nc.vector.reciprocal(rec[:], dn[:, 0:NSEG])
                sc = wk.tile([P, NSEG], f32, tag="sc")
                nc.vector.tensor_tensor(
                    sc[:], dn[:, NSEG : 2 * NSEG], rec[:], op=Alu.mult
                )

                # out = v * scale (broadcast over each 64-wide segment)
                ot = io.tile([P, FD], f32, tag="o")
                sc_b = sc[:].rearrange(
                    "p (s one) -> p s one", one=1
                ).broadcast_to([P, NSEG, D])
                nc.vector.tensor_tensor(
                    ot[:].rearrange("p (s d) -> p s d", d=D),
                    vt.rearrange("p (s d) -> p s d", d=D),
                    sc_b,
                    op=Alu.mult,
                )
                rows = slice(ch * JT * P, (ch + 1) * JT * P)
                ov = o_d[rows, :].rearrange("(j p) c -> p j c", p=P)
                nc.sync.dma_start(ov, ot[:].rearrange("p (j c) -> p j c", c=C))

    nc.compile()
    return nc


def get_nc():
    if "nc" not in _CACHE:
        _CACHE["nc"] = _build_nc()
    return _CACHE["nc"]


def shard_inputs(q, k, v):
    """core c -> (b = c//4, heads 2*(c%4), 2*(c%4)+1); returns list of in_maps."""
    maps = []
    for c in range(8):
        b, hp = divmod(c, 4)
        hs = slice(2 * hp, 2 * hp + 2)
        maps.append(
            {
                "q": np.ascontiguousarray(q[b, :, hs, :].reshape(T, C)),
                "k": np.ascontiguousarray(k[b, :, hs, :].reshape(T, C)),
                "v": np.ascontiguousarray(v[b, :, hs, :].reshape(T, C)),
            }
        )
    return maps


def gather_outputs(results):
    out = np.empty((B, T, H, D), dtype=np.float32)
    for c in range(8):
        b, hp = divmod(c, 4)
        out[b, :, 2 * hp : 2 * hp + 2, :] = results[c]["o"].reshape(T, HPC, D)
    return out


def kernel(q, k, v):
    from concourse.bass_utils import run_bass_kernel_spmd

    q = np.asarray(q, dtype=np.float32)
    k = np.asarray(k, dtype=np.float32)
    v = np.asarray(v, dtype=np.float32)
    nc = get_nc()
    maps = shard_inputs(q, k, v)
    res = run_bass_kernel_spmd(nc, maps, list(range(8)))
    return gather_outputs(res.results)


# revision 16
# speedup vs baseline: 2.2501x; 1.0178x over previous
"""Trainium2 Bass kernel for nn_CausalLinearSelfAttention_30013231464545.

Math note: the reference cumsums the [B,T,H,D,M] kv tensor over axis=-2,
which is the *D* axis (faithful to the original torch code), so
  kv_sum[b,t,h,d,m] = csD(kf)[b,t,h,d] * v[b,t,h,m]
and the whole module collapses to
  out[b,t,h,m] = (s / denom) * v[b,t,h,m]
with
  denom[b,t,h] = sum_d qf * cumsum_T(kf)      (true causal running key sum)
  s[b,t,h]     = sum_d qf * cumsum_D(kf)      (per-timestep D-prefix sum)
  qf = elu(q)+1 = min(exp(q), 1) + relu(q),  kf likewise.

Sharding: B*H = 16 (b,h) slices; each core takes one (b, head-pair) slice
[T=2048, 2*64] so DMA rows are 512B contiguous. No cross-core comm.

Per-core dataflow, v2 (T on partitions, (j,h,d) on free; fp16 on-chip):
  - 2 half-tensor chunks pipelined against the input DMA stream
  - feature maps: ACT exp (scalar) + fused (min 1) add relu via one
    scalar_tensor_tensor (DVE)
  - cumsum over T: per-tile triangular matmuls; the inter-tile carry runs
    entirely on the tensor engine (colsum matmuls onto partitions,
    one exclusive-tri16 matmul, rank-1 base adds) - no serial DVE chain
  - scalar engine copies PSUM ks -> SBUF f16 so the dot mults stay in
    DVE 2x fast mode
  - cumsum over D: one segmented scan per chunk (GpSimd, off the DVE)
  - dots over D: two f16 TT mults + one multi-axis tensor_reduce
  - out = v * (s/denom) broadcast multiply, DMA'd out per chunk
"""

import numpy as np
import sys

sys.path.insert(0, "/opt/trn_rl_repo")

B, T, H, D = 2, 2048, 8, 64
P = 128          # partitions (t per tile)
HPC = 2          # heads per core
C = HPC * D      # per-core free width = 128
NT = T // P      # 16 t-tiles per core
NCH = 2          # pipeline chunks
JT = NT // NCH   # 8 tiles per chunk
FD = JT * C      # 1024 free elements per chunk
NSEG = JT * HPC  # 16 (tile, head) segments per chunk

USE_GPSIMD_SCAN = False  # compiler rejects TensorTensorScan on the Pool engine

_CACHE = {}


def _build_nc():
    import concourse.bass as bass
    import concourse.bacc as bacc
    import concourse.mybir as mybir
    from concourse import tile

    dt = mybir.dt
    f32 = dt.float32
    f16 = dt.float16
    Alu = mybir.AluOpType
    Act = mybir.ActivationFunctionType

    nc = bacc.Bacc(None)

    q_d = nc.declare_dram_parameter("q", [T, C], f32, isOutput=False)
    k_d = nc.declare_dram_parameter("k", [T, C], f32, isOutput=False)
    v_d = nc.declare_dram_parameter("v", [T, C], f32, isOutput=False)
    o_d = nc.declare_dram_parameter("o", [T, C], f32, isOutput=True)

    # tri[t', t] = 1 if t' <= t  (lhsT for in-tile cumsum over partitions)
    tri_d = nc.inline_tensor(
        np.triu(np.ones((P, P), dtype=np.float16)), name="tri_const"
    )
    # gate2[p, (j, m)] = 1 if p < j: lhsT that sums colsum rows < j into
    # a broadcast over all 128 output partitions (the inter-tile base add)
    gate2_np = np.zeros((NT, NT, P), dtype=np.float16)
    for j in range(NT):
        gate2_np[:j, j, :] = 1.0
    gate2_d = nc.inline_tensor(gate2_np.reshape(NT, NT * P), name="gate2_const")
    # colsel[:, j, m] = (m == j): drops tile j's colsum onto psum row j
    colsel_np = np.zeros((P, JT, JT), dtype=np.float16)
    for j in range(JT):
        colsel_np[:, j, j] = 1.0
    colsel_d = nc.inline_tensor(colsel_np.reshape(P, JT * JT), name="colsel_const")

    with tile.TileContext(nc) as tc:
        with (
            tc.tile_pool(name="const", bufs=1) as cpool,
            tc.tile_pool(name="io", bufs=2) as io,
            tc.tile_pool(name="wk", bufs=2) as wk,
            tc.tile_pool(name="ps", bufs=2, space="PSUM") as pp,
            tc.tile_pool(name="pcs", bufs=1, space="PSUM") as pcs,
        ):
            # ---- tiny consts first so matmuls are never DMA-gated ----
            tri_t = cpool.tile([P, P], f16, tag="tri")
            nc.sync.dma_start(tri_t[:], tri_d[:])
            gate2_t = cpool.tile([NT, NT * P], f16, tag="gate2")
            nc.sync.dma_start(gate2_t[:], gate2_d[:])
            colsel_t = cpool.tile([P, JT * JT], f16, tag="colsel")
            nc.sync.dma_start(colsel_t[:], colsel_d[:])

            # ---- input DMAs: chunked, in consumption order ----
            qtw = io.tile([P, T], f32, tag="q")
            ktw = io.tile([P, T], f32, tag="k")
            vtw = io.tile([P, T], f32, tag="v")

            def load_chunk(tw, d_, ch):
                rows = slice(ch * JT * P, (ch + 1) * JT * P)
                nc.sync.dma_start(
                    tw[:, ch * FD : (ch + 1) * FD].rearrange(
                        "p (j c) -> p j c", c=C
                    ),
                    d_[rows, :].rearrange("(j p) c -> p j c", p=P),
                )

            for ch in range(NCH):
                load_chunk(ktw, k_d, ch)
                load_chunk(qtw, q_d, ch)
            for ch in range(NCH):
                load_chunk(vtw, v_d, ch)
            # segmented-scan reset mask: 0 at the first column of each 64-seg
            mask_t = cpool.tile([P, FD], f16, tag="mask")
            nc.vector.memset(mask_t[:], 1.0)
            nc.vector.memset(
                mask_t[:].rearrange("p (s d) -> p s d", d=D)[:, :, 0:1], 0.0
            )
            # per-tile colsum rows for chunk>0, spread over partitions
            csm = cpool.tile([NT, C], f16, tag="csm")
            cst = [None] * NCH

            scan_eng = nc.gpsimd if USE_GPSIMD_SCAN else nc.vector

            for ch in range(NCH):
                gsl = slice(ch * FD, (ch + 1) * FD)
                kt = ktw[:, gsl]
                qt = qtw[:, gsl]
                vt = vtw[:, gsl]

                # ---- feature maps: f = min(exp(x),1) + relu(x) ----
                ek = wk.tile([P, FD], f16, tag="ek")
                nc.scalar.activation(ek[:], kt, Act.Exp)
                rk = wk.tile([P, FD], f16, tag="rk")
                nc.vector.tensor_scalar_max(rk[:], kt, 0.0)
                kf = wk.tile([P, FD], f16, tag="kf")
                nc.vector.scalar_tensor_tensor(
                    kf[:], ek[:], 1.0, rk[:], op0=Alu.min, op1=Alu.add
                )

                eq = wk.tile([P, FD], f16, tag="eq")
                nc.scalar.activation(eq[:], qt, Act.Exp)
                rq = wk.tile([P, FD], f16, tag="rq")
                nc.scalar.activation(rq[:], qt, Act.Relu)
                qf = wk.tile([P, FD], f16, tag="qf")
                nc.vector.scalar_tensor_tensor(
                    qf[:], eq[:], 1.0, rq[:], op0=Alu.min, op1=Alu.add
                )

                # ---- per-tile colsums onto psum rows 0..JT-1 ----
                cs_ps = pcs.tile([JT, C], f32, tag="cs")
                for j in range(JT):
                    nc.tensor.matmul(
                        cs_ps[:],
                        colsel_t[:, j * JT : (j + 1) * JT],
                        kf[:, j * C : (j + 1) * C],
                        start=(j == 0), stop=(j == JT - 1),
                    )
                cst_ch = wk.tile([JT, C], f16, tag=f"cst{ch}")
                cst[ch] = cst_ch
                nc.scalar.copy(cst_ch[:], cs_ps[:])
                # stage rows into csm (only chunk>0 reads it)
                nc.sync.dma_start(csm[ch * JT : (ch + 1) * JT, :], cst[ch][:])

                # ---- cumsum over T into PSUM: tri matmuls + per-tile
                # gated-colsum base matmuls (start=True resets the whole
                # PSUM bank, so only the first write per bank may set it)
                nk = (ch + 1) * JT
                cs_src = cst[0] if ch == 0 else csm
                ks = pp.tile([P, FD], f32, tag="ks")
                for j in range(JT):
                    sl = slice(j * C, (j + 1) * C)
                    nc.tensor.matmul(
                        ks[:, sl], tri_t[:], kf[:, sl],
                        start=(j % 4 == 0), stop=False,
                    )
                ksf = wk.tile([P, FD], f16, tag="ksf")
                for half in range(FD // 512):
                    for j in range(half * 4, half * 4 + 4):
                        jg = ch * JT + j
                        sl = slice(j * C, (j + 1) * C)
                        if jg == 0:
                            continue
                        nc.tensor.matmul(
                            ks[:, sl],
                            gate2_t[0:nk, jg * P : (jg + 1) * P],
                            cs_src[0:nk, :],
                            start=False,
                            stop=(j % 4 == 3),
                        )
                    hs = slice(half * 512, (half + 1) * 512)
                    nc.scalar.copy(ksf[:, hs], ks[:, hs])

                # ---- cumsum over D: one segmented scan ----
                csDm = wk.tile([P, FD], f16, tag="csDm")
                scan_eng.tensor_tensor_scan(
                    csDm[:], mask_t[:], kf[:], 0.0, op0=Alu.mult, op1=Alu.add
                )

                # ---- dots over D (s-part first: ready before ksf) ----
                scr = wk.tile([P, 2 * FD], f16, tag="scr")
                dn = wk.tile([P, 2 * NSEG], f32, tag="dn")
                nc.vector.tensor_tensor(
                    scr[:, FD : 2 * FD], qf[:], csDm[:], op=Alu.mult
                )
                nc.vector.tensor_tensor(
                    scr[:, 0:FD], qf[:], ksf[:], op=Alu.mult
                )
                nc.vector.tensor_reduce(
                    dn[:],
                    scr[:].rearrange("p (s d) -> p s d", d=D),
                    axis=mybir.AxisListType.X,
                    op=Alu.add,
                )

                # scale = s / denom
                rec = wk.tile([P, NSEG], f32, tag="rec")
                nc.vector.reciprocal(rec[:], dn[:, 0:NSEG])
                sc = wk.tile([P, NSEG], f32, tag="sc")
                nc.vector.tensor_tensor(
                    sc[:], dn[:, NSEG : 2 * NSEG], rec[:], op=Alu.mult
                )

                # out = v * scale (broadcast over each 64-wide segment)
                ot = io.tile([P, FD], f32, tag="o")
                sc_b = sc[:].rearrange(
                    "p (s one) -> p s one", one=1
                ).broadcast_to([P, NSEG, D])
                nc.vector.tensor_tensor(
                    ot[:].rearrange("p (s d) -> p s d", d=D),
                    vt.rearrange("p (s d) -> p s d", d=D),
                    sc_b,
                    op=Alu.mult,
                )
                rows = slice(ch * JT * P, (ch + 1) * JT * P)
                ov = o_d[rows, :].rearrange("(j p) c -> p j c", p=P)
                nc.sync.dma_start(ov, ot[:].rearrange("p (j c) -> p j c", c=C))

    nc.compile()
    return nc


def get_nc():
    if "nc" not in _CACHE:
        _CACHE["nc"] = _build_nc()
    return _CACHE["nc"]


def shard_inputs(q, k, v):
    """core c -> (b = c//4, heads 2*(c%4), 2*(c%4)+1); returns list of in_maps."""
    maps = []
    for c in range(8):
        b, hp = divmod(c, 4)
        hs = slice(2 * hp, 2 * hp + 2)
        maps.append(
            {
                "q": np.ascontiguousarray(q[b, :, hs, :].reshape(T, C)),
                "k": np.ascontiguousarray(k[b, :, hs, :].reshape(T, C)),
                "v": np.ascontiguousarray(v[b, :, hs, :].reshape(T, C)),
            }
        )
    return maps


def gather_outputs(results):
    out = np.empty((B, T, H, D), dtype=np.float32)
    for c in range(8):
        b, hp = divmod(c, 4)
        out[b, :, 2 * hp : 2 * hp + 2, :] = results[c]["o"].reshape(T, HPC, D)
    return out


def kernel(q, k, v):
    from concourse.bass_utils import run_bass_kernel_spmd

    q = np.asarray(q, dtype=np.float32)
    k = np.asarray(k, dtype=np.float32)
    v = np.asarray(v, dtype=np.float32)
    nc = get_nc()
    maps = shard_inputs(q, k, v)
    res = run_bass_kernel_spmd(nc, maps, list(range(8)))
    return gather_outputs(res.results)


# revision 18
# speedup vs baseline: 2.2541x; 1.0018x over previous
"""Trainium2 Bass kernel for nn_CausalLinearSelfAttention_30013231464545.

Math note: the reference cumsums the [B,T,H,D,M] kv tensor over axis=-2,
which is the *D* axis (faithful to the original torch code), so
  kv_sum[b,t,h,d,m] = csD(kf)[b,t,h,d] * v[b,t,h,m]
and the whole module collapses to
  out[b,t,h,m] = (s / denom) * v[b,t,h,m]
with
  denom[b,t,h] = sum_d qf * cumsum_T(kf)      (true causal running key sum)
  s[b,t,h]     = sum_d qf * cumsum_D(kf)      (per-timestep D-prefix sum)
  qf = elu(q)+1 = min(exp(q), 1) + relu(q),  kf likewise.

Sharding: B*H = 16 (b,h) slices; each core takes one (b, head-pair) slice
[T=2048, 2*64] so DMA rows are 512B contiguous. No cross-core comm.

Per-core dataflow, v2 (T on partitions, (j,h,d) on free; fp16 on-chip):
  - 2 half-tensor chunks pipelined against the input DMA stream
  - feature maps: ACT exp (scalar) + fused (min 1) add relu via one
    scalar_tensor_tensor (DVE)
  - cumsum over T: per-tile triangular matmuls; the inter-tile carry runs
    entirely on the tensor engine (colsum matmuls onto partitions,
    one exclusive-tri16 matmul, rank-1 base adds) - no serial DVE chain
  - scalar engine copies PSUM ks -> SBUF f16 so the dot mults stay in
    DVE 2x fast mode
  - cumsum over D: one segmented scan per chunk (GpSimd, off the DVE)
  - dots over D: two f16 TT mults + one multi-axis tensor_reduce
  - out = v * (s/denom) broadcast multiply, DMA'd out per chunk
"""

import numpy as np
import sys

sys.path.insert(0, "/opt/trn_rl_repo")

B, T, H, D = 2, 2048, 8, 64
P = 128          # partitions (t per tile)
HPC = 2          # heads per core
C = HPC * D      # per-core free width = 128
NT = T // P      # 16 t-tiles per core
NCH = 2          # pipeline chunks
JT = NT // NCH   # 8 tiles per chunk
FD = JT * C      # 1024 free elements per chunk
NSEG = JT * HPC  # 16 (tile, head) segments per chunk

USE_GPSIMD_SCAN = False  # compiler rejects TensorTensorScan on the Pool engine

_CACHE = {}


def _build_nc():
    import concourse.bass as bass
    import concourse.bacc as bacc
    import concourse.mybir as mybir
    from concourse import tile

    dt = mybir.dt
    f32 = dt.float32
    f16 = dt.float16
    Alu = mybir.AluOpType
    Act = mybir.ActivationFunctionType

    nc = bacc.Bacc(None)

    q_d = nc.declare_dram_parameter("q", [T, C], f32, isOutput=False)
    k_d = nc.declare_dram_parameter("k", [T, C], f32, isOutput=False)
    v_d = nc.declare_dram_parameter("v", [T, C], f32, isOutput=False)
    o_d = nc.declare_dram_parameter("o", [T, C], f32, isOutput=True)

    # tri[t', t] = 1 if t' <= t  (lhsT for in-tile cumsum over partitions)
    tri_d = nc.inline_tensor(
        np.triu(np.ones((P, P), dtype=np.float16)), name="tri_const"
    )
    # gate2[p, (j, m)] = 1 if p < j: lhsT that sums colsum rows < j into
    # a broadcast over all 128 output partitions (the inter-tile base add)
    gate2_np = np.zeros((NT, NT, P), dtype=np.float16)
    for j in range(NT):
        gate2_np[:j, j, :] = 1.0
    gate2_d = nc.inline_tensor(gate2_np.reshape(NT, NT * P), name="gate2_const")
    # colsel[:, j, m] = (m == j): drops tile j's colsum onto psum row j
    colsel_np = np.zeros((P, JT, JT), dtype=np.float16)
    for j in range(JT):
        colsel_np[:, j, j] = 1.0
    colsel_d = nc.inline_tensor(colsel_np.reshape(P, JT * JT), name="colsel_const")

    with tile.TileContext(nc) as tc:
        with (
            tc.tile_pool(name="const", bufs=1) as cpool,
            tc.tile_pool(name="io", bufs=2) as io,
            tc.tile_pool(name="wk", bufs=2) as wk,
            tc.tile_pool(name="ps", bufs=2, space="PSUM") as pp,
            tc.tile_pool(name="pcs", bufs=1, space="PSUM") as pcs,
        ):
            # ---- DMA order: tri, then chunk-0 inputs, then the rest ----
            tri_t = cpool.tile([P, P], f16, tag="tri")
            nc.sync.dma_start(tri_t[:], tri_d[:])

            qtw = io.tile([P, T], f32, tag="q")
            ktw = io.tile([P, T], f32, tag="k")
            vtw = io.tile([P, T], f32, tag="v")

            def load_chunk(tw, d_, ch):
                rows = slice(ch * JT * P, (ch + 1) * JT * P)
                nc.sync.dma_start(
                    tw[:, ch * FD : (ch + 1) * FD].rearrange(
                        "p (j c) -> p j c", c=C
                    ),
                    d_[rows, :].rearrange("(j p) c -> p j c", p=P),
                )

            load_chunk(ktw, k_d, 0)
            load_chunk(qtw, q_d, 0)
            colsel_t = cpool.tile([P, JT * JT], f16, tag="colsel")
            nc.sync.dma_start(colsel_t[:], colsel_d[:])
            gate2_t = cpool.tile([NT, NT * P], f16, tag="gate2")
            nc.sync.dma_start(gate2_t[:], gate2_d[:])
            load_chunk(ktw, k_d, 1)
            load_chunk(qtw, q_d, 1)
            load_chunk(vtw, v_d, 0)
            load_chunk(vtw, v_d, 1)
            # segmented-scan reset mask: 0 at the first column of each 64-seg
            mask_t = cpool.tile([P, FD], f16, tag="mask")
            nc.gpsimd.memset(mask_t[:], 1.0)
            nc.gpsimd.memset(
                mask_t[:].rearrange("p (s d) -> p s d", d=D)[:, :, 0:1], 0.0
            )
            # per-tile colsum rows for chunk>0, spread over partitions
            csm = cpool.tile([NT, C], f16, tag="csm")
            cst = [None] * NCH

            scan_eng = nc.gpsimd if USE_GPSIMD_SCAN else nc.vector

            for ch in range(NCH):
                gsl = slice(ch * FD, (ch + 1) * FD)
                kt = ktw[:, gsl]
                qt = qtw[:, gsl]
                vt = vtw[:, gsl]

                # ---- feature maps: f = min(exp(x),1) + relu(x) ----
                ek = wk.tile([P, FD], f16, tag="ek")
                nc.scalar.activation(ek[:], kt, Act.Exp)
                rk = wk.tile([P, FD], f16, tag="rk")
                nc.scalar.activation(rk[:], kt, Act.Relu)
                mek = wk.tile([P, FD], f16, tag="mek")
                nc.vector.tensor_scalar_min(mek[:], ek[:], 1.0)
                kf = wk.tile([P, FD], f16, tag="kf")
                nc.vector.tensor_tensor(kf[:], mek[:], rk[:], op=Alu.add)

                eq = wk.tile([P, FD], f16, tag="eq")
                nc.scalar.activation(eq[:], qt, Act.Exp)
                rq = wk.tile([P, FD], f16, tag="rq")
                nc.scalar.activation(rq[:], qt, Act.Relu)
                meq = wk.tile([P, FD], f16, tag="meq")
                nc.vector.tensor_scalar_min(meq[:], eq[:], 1.0)
                qf = wk.tile([P, FD], f16, tag="qf")
                nc.vector.tensor_tensor(qf[:], meq[:], rq[:], op=Alu.add)

                # ---- per-tile colsums onto psum rows 0..JT-1 ----
                cs_ps = pcs.tile([JT, C], f32, tag="cs")
                for j in range(JT):
                    nc.tensor.matmul(
                        cs_ps[:],
                        colsel_t[:, j * JT : (j + 1) * JT],
                        kf[:, j * C : (j + 1) * C],
                        start=(j == 0), stop=(j == JT - 1),
                    )
                cst_ch = wk.tile([JT, C], f16, tag=f"cst{ch}")
                cst[ch] = cst_ch
                nc.scalar.copy(cst_ch[:], cs_ps[:])
                # stage rows into csm (only chunk>0 reads it)
                nc.sync.dma_start(csm[ch * JT : (ch + 1) * JT, :], cst[ch][:])

                # ---- cumsum over T into PSUM: tri matmuls + per-tile
                # gated-colsum base matmuls (start=True resets the whole
                # PSUM bank, so only the first write per bank may set it)
                nk = (ch + 1) * JT
                cs_src = cst[0] if ch == 0 else csm
                ks = pp.tile([P, FD], f32, tag="ks")
                for j in range(JT):
                    sl = slice(j * C, (j + 1) * C)
                    nc.tensor.matmul(
                        ks[:, sl], tri_t[:], kf[:, sl],
                        start=(j % 4 == 0), stop=False,
                    )
                ksf = wk.tile([P, FD], f16, tag="ksf")
                for half in range(FD // 512):
                    for j in range(half * 4, half * 4 + 4):
                        jg = ch * JT + j
                        sl = slice(j * C, (j + 1) * C)
                        if jg == 0:
                            continue
                        nc.tensor.matmul(
                            ks[:, sl],
                            gate2_t[0:nk, jg * P : (jg + 1) * P],
                            cs_src[0:nk, :],
                            start=False,
                            stop=(j % 4 == 3),
                        )
                    hs = slice(half * 512, (half + 1) * 512)
                    nc.scalar.copy(ksf[:, hs], ks[:, hs])

                # ---- cumsum over D: one segmented scan ----
                csDm = wk.tile([P, FD], f16, tag="csDm")
                scan_eng.tensor_tensor_scan(
                    csDm[:], mask_t[:], kf[:], 0.0, op0=Alu.mult, op1=Alu.add
                )

                # ---- dots over D (s-part first: ready before ksf) ----
                scr = wk.tile([P, 2 * FD], f16, tag="scr")
                dn = wk.tile([P, 2 * NSEG], f32, tag="dn")
                nc.vector.tensor_tensor(
                    scr[:, FD : 2 * FD], qf[:], csDm[:], op=Alu.mult
                )
                nc.vector.tensor_tensor(
                    scr[:, 0:FD], qf[:], ksf[:], op=Alu.mult
                )
                nc.vector.tensor_reduce(
                    dn[:],
                    scr[:].rearrange("p (s d) -> p s d", d=D),
                    axis=mybir.AxisListType.X,
                    op=Alu.add,
                )

                # scale = s / denom
                rec = wk.tile([P, NSEG], f32, tag="rec")
                nc.vector.reciprocal(rec[:], dn[:, 0:NSEG])
                sc = wk.tile([P, NSEG], f32, tag="sc")
                nc.vector.tensor_tensor(
                    sc[:], dn[:, NSEG : 2 * NSEG], rec[:], op=Alu.mult
                )

                # out = v * scale (broadcast over each 64-wide segment)
                ot = io.tile([P, FD], f32, tag="o")
                sc_b = sc[:].rearrange(
                    "p (s one) -> p s one", one=1
                ).broadcast_to([P, NSEG, D])
                nc.vector.tensor_tensor(
                    ot[:].rearrange("p (s d) -> p s d", d=D),
                    vt.rearrange("p (s d) -> p s d", d=D),
                    sc_b,
                    op=Alu.mult,
                )
                rows = slice(ch * JT * P, (ch + 1) * JT * P)
                ov = o_d[rows, :].rearrange("(j p) c -> p j c", p=P)
                nc.sync.dma_start(ov, ot[:].rearrange("p (j c) -> p j c", c=C))

    nc.compile()
    return nc


def get_nc():
    if "nc" not in _CACHE:
        _CACHE["nc"] = _build_nc()
    return _CACHE["nc"]


def shard_inputs(q, k, v):
    """core c -> (b = c//4, heads 2*(c%4), 2*(c%4)+1); returns list of in_maps."""
    maps = []
    for c in range(8):
        b, hp = divmod(c, 4)
        hs = slice(2 * hp, 2 * hp + 2)
        maps.append(
            {
                "q": np.ascontiguousarray(q[b, :, hs, :].reshape(T, C)),
                "k": np.ascontiguousarray(k[b, :, hs, :].reshape(T, C)),
                "v": np.ascontiguousarray(v[b, :, hs, :].reshape(T, C)),
            }
        )
    return maps


def gather_outputs(results):
    out = np.empty((B, T, H, D), dtype=np.float32)
    for c in range(8):
        b, hp = divmod(c, 4)
        out[b, :, 2 * hp : 2 * hp + 2, :] = results[c]["o"].reshape(T, HPC, D)
    return out


def kernel(q, k, v):
    from concourse.bass_utils import run_bass_kernel_spmd

    q = np.asarray(q, dtype=np.float32)
    k = np.asarray(k, dtype=np.float32)
    v = np.asarray(v, dtype=np.float32)
    nc = get_nc()
    maps = shard_inputs(q, k, v)
    res = run_bass_kernel_spmd(nc, maps, list(range(8)))
    return gather_outputs(res.results)
